# revision 3
# baseline (speedup 1.0000x reference)
"""Trainium2 Bass kernel for CompositionalCodebookLayer (vq_codebook).

Problem: x (4, 2048, 2048) f32, codebook (8, 2048, 256) f32.
For each token (B*S = 8192) and each of the 8 codebooks, find the nearest
code (argmin squared euclidean distance over 2048 codes) and emit the
snapped vectors (gathered codebook rows) + indices.

Sharding: expert-style — core c owns codebook c and processes all 8192
tokens for it.  Per-core device program:

  * distances via 3-term fp16-split matmul (x·c = xh·ch + xh·cl + xl·ch,
    each product exact in fp32 PSUM; |err| ~1e-5, below fp32-reference
    noise) — dots land in PSUM [128 tok, 2048 codes] with codes in
    DESCENDING id order,
  * single-pass fused argmax on the Vector engine via a custom DVE op:
    body streams s = dots - 0.5*||c||², tracks the running max with an
    inclusive scan, and accum-maxes the stream position of the last
    running-max event.  With the descending code order that position is
    exactly the FIRST code id achieving the max (jnp.argmin tie rule),
  * per-token gather of the winning codebook rows with an indirect DMA
    (per-partition offsets), written straight to the per-core output.

The host wrapper splits/casts/transposes inputs per core, runs the SPMD
program on cores 0-7 via run_bass_kernel_spmd, and reassembles the full
(out, ids) pair.
"""

import numpy as np

import concourse.bass as bass
import concourse.bacc as bacc
import concourse.mybir as mybir
import concourse.tile as tile
import concourse.dve_ops as dve_ops_mod
from concourse.bass_utils import run_bass_kernel_spmd
from concourse.dve_ops import DveOp
from concourse.dve_spec import (
    AluOp,
    Idx,
    Spec,
    Src0,
    Src1,
    Zero,
    eq,
    lower,
    scan,
    select,
    _has_src1,
)
from concourse.dve_table_gen import dve_ver_for
from concourse.dve_uop import DveOpSpec

B, S, DIM = 4, 2048, 2048
C, N = 8, 2048
D = DIM // C          # 256
T = B * S             # 8192 tokens
TT = T // 128         # 64 token tiles
GB = 8                # gather batch (token tiles per indirect DMA)


# --------------------------------------------------------------------------
# custom DVE op: single-pass biased argmax (see module docstring)
# --------------------------------------------------------------------------
def _ref_argmax1p(in0, in1, s0, s1, imm2):
    sub = in0.astype(np.float32) - in1.astype(np.float32)
    f = sub.reshape(sub.shape[0], -1)
    r = np.maximum.accumulate(f, axis=-1)
    ev = f == r
    idx = np.arange(f.shape[-1], dtype=np.float32)[None, :]
    body = np.where(ev, idx, 0.0).astype(np.float32)
    acc = body.max(axis=-1, keepdims=True)
    return body.reshape(in0.shape), acc


def make_argmax_op():
    name = "SUB_ARGMAX_SCAN_ANT"
    for op in dve_ops_mod.OPS:
        if op.name == name:
            return op
    sub = Src0 - Src1
    r = scan(AluOp.MAX, sub)
    spec = Spec(
        body=select(eq(sub, r), Idx, Zero),
        accum=AluOp.MAX,
        reference=_ref_argmax1p,
    )
    ver = dve_ver_for("TRN2")
    uops = lower(spec, ver=ver)
    sha = DveOpSpec(name=name, uops=uops, rd1_en=_has_src1(spec)).sha(ver)
    op = DveOp(name, spec, subdim=False, uops_sha={ver: sha})
    dve_ops_mod.OPS.append(op)
    dve_ops_mod.CUSTOM_DVE_SPECS[name] = op.spec
    dve_ops_mod._SUB_OPCODE_FOR_NAME[name] = (
        dve_ops_mod._CUSTOM_DVE_ROW_BASE + len(dve_ops_mod.OPS) - 1
    )
    assert dve_ops_mod._SUB_OPCODE_FOR_NAME[name] < 0x20
    return op


# --------------------------------------------------------------------------
# device program (identical on all 8 cores; per-core data differs)
# --------------------------------------------------------------------------
_NC_CACHE = {}


def build_nc():
    if "nc" in _NC_CACHE:
        return _NC_CACHE["nc"]
    op = make_argmax_op()
    f16 = mybir.dt.float16
    f32 = mybir.dt.float32
    nc = bacc.Bacc("TRN2", target_bir_lowering=False, debug=False)
    xhiT_d = nc.dram_tensor("xhiT", [D, T], f16, kind="ExternalInput")
    xloT_d = nc.dram_tensor("xloT", [D, T], f16, kind="ExternalInput")
    chiT_d = nc.dram_tensor("chiT", [D, N], f16, kind="ExternalInput")
    cloT_d = nc.dram_tensor("cloT", [D, N], f16, kind="ExternalInput")
    c2h_d = nc.dram_tensor("c2h", [1, N], f32, kind="ExternalInput")
    cb_d = nc.dram_tensor("cb", [N, D], f32, kind="ExternalInput")
    out_d = nc.dram_tensor("outc", [TT, 128, D], f32, kind="ExternalOutput")
    ids_d = nc.dram_tensor("idsc", [128, TT], mybir.dt.int32, kind="ExternalOutput")

    with tile.TileContext(nc) as tc:
        with (
            tc.tile_pool(name="tables", bufs=1) as tables,
            tc.tile_pool(name="xst", bufs=3) as xst,
            tc.tile_pool(name="psum", bufs=2, space="PSUM") as psum,
            tc.tile_pool(name="work", bufs=1) as work,
            tc.tile_pool(name="gat", bufs=2) as gat,
            tc.tile_pool(name="small", bufs=2) as small,
        ):
            chi = tables.tile([128, 2, N], f16)
            clo = tables.tile([128, 2, N], f16)
            nc.sync.dma_start(chi[:], chiT_d.ap().rearrange("(k p) t -> p k t", k=2))
            nc.sync.dma_start(clo[:], cloT_d.ap().rearrange("(k p) t -> p k t", k=2))
            c2rep = tables.tile([128, N], f32)
            src = c2h_d.ap()
            nc.sync.dma_start(
                c2rep[:], bass.AP(src.tensor, src.offset, [[0, 128]] + src.ap[1:])
            )
            scratch = work.tile([128, N], f32)
            accs = tables.tile([128, TT], f32)
            ids_all = tables.tile([128, TT], mybir.dt.int32)

            XB = 4  # token tiles per x staging DMA
            for g in range(TT // GB):        # gather batches of GB token tiles
                for sb in range(GB // XB):   # x staging chunks
                    xhi = xst.tile([128, 2, XB * 128], f16, tag="xhi")
                    xlo = xst.tile([128, 2, XB * 128], f16, tag="xlo")
                    t0 = (g * GB + sb * XB) * 128
                    nc.sync.dma_start(
                        xhi[:],
                        xhiT_d.ap()[:, t0 : t0 + XB * 128].rearrange(
                            "(k p) t -> p k t", k=2
                        ),
                    )
                    nc.sync.dma_start(
                        xlo[:],
                        xloT_d.ap()[:, t0 : t0 + XB * 128].rearrange(
                            "(k p) t -> p k t", k=2
                        ),
                    )
                    for u in range(XB):      # token tiles within the chunk
                        tt = g * GB + sb * XB + u
                        ps = psum.tile([128, N], f32, tag="ps")
                        lhs_list = (
                            (xhi, 0, (chi, clo)),
                            (xhi, 1, (chi, clo)),
                            (xlo, 0, (chi,)),
                            (xlo, 1, (chi,)),
                        )
                        n_total = 4
                        for li, (lhs, k, rhss) in enumerate(lhs_list):
                            for rhs in rhss:
                                for n in range(n_total):
                                    nc.tensor.matmul(
                                        ps[:, n * 512 : (n + 1) * 512],
                                        lhs[:, k, u * 128 : (u + 1) * 128],
                                        rhs[:, k, n * 512 : (n + 1) * 512],
                                        start=(li == 0 and rhs is chi),
                                        stop=(li == 3),
                                    )
                        nc.vector._custom_dve(
                            op,
                            out=scratch[:],
                            in0=ps[:],
                            in1=c2rep[:],
                            accum_out=accs[:, tt : tt + 1],
                        )
                # ids for this gather batch: code id = (N-1) - stream pos
                ids_f = small.tile([128, GB], f32, tag="idsf")
                nc.vector.tensor_scalar(
                    out=ids_f[:],
                    in0=accs[:, g * GB : (g + 1) * GB],
                    scalar1=-1.0,
                    scalar2=float(N - 1),
                    op0=mybir.AluOpType.mult,
                    op1=mybir.AluOpType.add,
                )
                ids_i = ids_all[:, g * GB : (g + 1) * GB]
                nc.vector.tensor_copy(ids_i, ids_f[:])
                gath = gat.tile([128, GB, D], f32, tag="gath")
                for u in range(GB):
                    nc.gpsimd.indirect_dma_start(
                        out=gath[:, u, :],
                        out_offset=None,
                        in_=cb_d.ap(),
                        in_offset=bass.IndirectOffsetOnAxis(
                            ap=ids_i[:, u : u + 1], axis=0
                        ),
                    )
                nc.sync.dma_start(
                    out_d.ap()[g * GB : (g + 1) * GB].rearrange("g p d -> p g d"),
                    gath[:],
                )
            nc.sync.dma_start(ids_d.ap(), ids_all[:])
    nc.compile()
    _NC_CACHE["nc"] = nc
    return nc


# --------------------------------------------------------------------------
# host wrapper
# --------------------------------------------------------------------------
def _prep_core(xc, cbc):
    """xc: [T, D] f32 slice for codebook c; cbc: [N, D] f32."""
    xh = xc.astype(np.float16)
    xl = (xc - xh.astype(np.float32)).astype(np.float16)
    ch = cbc.astype(np.float16)
    cl = (cbc - ch.astype(np.float32)).astype(np.float16)
    rev = np.arange(N - 1, -1, -1)
    c2h = (0.5 * np.sum(cbc.astype(np.float32) * cbc, axis=-1))[rev]
    return {
        "xhiT": np.ascontiguousarray(xh.T),
        "xloT": np.ascontiguousarray(xl.T),
        "chiT": np.ascontiguousarray(ch.T[:, rev]),
        "cloT": np.ascontiguousarray(cl.T[:, rev]),
        "c2h": np.ascontiguousarray(c2h[None, :]).astype(np.float32),
        "cb": np.ascontiguousarray(cbc),
    }


def run_device(x, codebook, trace=False):
    nc = build_nc()
    xr = np.asarray(x, dtype=np.float32).reshape(T, C, D)
    cb = np.asarray(codebook, dtype=np.float32)
    in_maps = [_prep_core(np.ascontiguousarray(xr[:, c, :]), cb[c]) for c in range(C)]
    res = run_bass_kernel_spmd(nc, in_maps, core_ids=list(range(C)), trace=trace)
    out = np.empty((T, C, D), dtype=np.float32)
    ids = np.empty((T, C), dtype=np.int32)
    for c in range(C):
        r = res.results[c]
        out[:, c, :] = r["outc"].reshape(T, D)
        ids[:, c] = r["idsc"].T.reshape(T)
    return out.reshape(B, S, DIM), ids.reshape(B, S, C), res


def kernel(x, codebook):
    out, ids, _ = run_device(x, codebook)
    return out, ids
